# revision 1
# baseline (speedup 1.0000x reference)
# Trainium2 Bass kernel for nn_DeepSphere (ChebConv GNN, K=6, 7 blocks + FC).
#
# Strategy: pure batch data-parallel over 8 NeuronCores (4 batch each).
# Per core, each ChebConv propagate is a padded-CSR gather (dma_gather,
# degree-sorted node permutation so per-stripe pads are tight), a DVE
# multiply by the Laplacian edge weights, and a PE identity-matmul
# segmented reduction accumulating in PSUM. The per-order channel mixing
# runs as an end-of-block pass: bf/fp16 transpose-gathers put (b, ci) on
# partitions and host-built block-diagonal weight matrices contract all
# batch elements in single matmuls. Activations live in HBM as fp16
# node-major tables; all accumulation is fp32 (PSUM).
import sys
import numpy as np

sys.path.insert(0, "/opt/trn_rl_repo")

EPS = 1e-5
F16 = np.float16

# ----------------------------------------------------------------------
# Host-side graph preprocessing
# ----------------------------------------------------------------------


def _effective_graph(src, dst, w, n):
    """Replicate JAX OOB semantics: segment_sum/scatter drop idx >= n,
    gather clamps. Returns (src_eff, dst, wl) for the kept edges."""
    src = np.asarray(src).astype(np.int64)
    dst = np.asarray(dst).astype(np.int64)
    w = np.asarray(w).astype(np.float32)
    deg = np.zeros(n, np.float32)
    m = src < n
    np.add.at(deg, src[m], w[m])
    dinv = np.where(deg > 0, 1.0 / np.sqrt(deg), 0.0).astype(np.float32)
    srcc = np.minimum(src, n - 1)
    dstc = np.minimum(dst, n - 1)
    wl = (-w * dinv[srcc] * dinv[dstc]).astype(np.float32)
    keep = dst < n
    srcc, dst, wl = srcc[keep], dst[keep], wl[keep]
    # edges whose (clamped) source is the hub row n-1 collapse to a rank-1
    # term out[n] += c_n * T[n-1]
    hub = srcc == (n - 1)
    c = np.zeros(n, np.float32)
    np.add.at(c, dst[hub], wl[hub])
    return srcc[~hub], dst[~hub], wl[~hub], c


def _wrap_idx_tile(idx):
    idx = np.asarray(idx)
    count = idx.shape[0]
    assert count % 16 == 0
    t = np.zeros((128, count // 16), np.int16)
    base = idx.reshape(count // 16, 16).T.astype(np.int16)
    for g in range(8):
        t[g * 16:(g + 1) * 16, :] = base
    return t


class _G:
    pass


def _build_graph_struct(src, dst, w, n, max_slots):
    g = _G()
    g.n = n
    Q = n // 128
    g.Q = Q
    src, dst, wl, hub_c = _effective_graph(src, dst, w, n)
    indeg = np.bincount(dst, minlength=n)
    node_of_rank = np.argsort(-indeg, kind="stable")
    r = np.arange(n)
    pos_of_rank = (r % 128) * Q + (r // 128)
    g.node_at_pos = np.empty(n, np.int64)
    g.node_at_pos[pos_of_rank] = node_of_rank
    g.pos_of_node = np.empty(n, np.int64)
    g.pos_of_node[g.node_at_pos] = np.arange(n)

    deg_by_rank = indeg[node_of_rank]
    pads = np.maximum(deg_by_rank.reshape(Q, 128).max(axis=1), 1).astype(np.int64)
    g.pads = pads
    g.slot_off = np.concatenate([[0], np.cumsum(pads)])
    g.S = int(g.slot_off[-1])

    order = np.argsort(dst, kind="stable")
    dst_sorted = dst[order]
    starts = np.searchsorted(dst_sorted, np.arange(n))
    ends = np.searchsorted(dst_sorted, np.arange(n) + 1)

    idx = np.zeros(g.S * 128, np.int64)
    wlg = np.zeros((128, g.S), np.float32)
    pos_src = g.pos_of_node[src]
    for rr in range(n):
        q, p = rr // 128, rr % 128
        node = node_of_rank[rr]
        es = order[starts[node]:ends[node]]
        s0 = g.slot_off[q]
        idx[(s0 + np.arange(len(es))) * 128 + p] = pos_src[es]
        wlg[p, s0:s0 + len(es)] = wl[es]
    g.idx_prop_tile = _wrap_idx_tile(idx)
    g.wl = wlg.astype(F16)
    g.wl2 = (2.0 * wlg).astype(F16)
    g.idx_eins_tile = _wrap_idx_tile(np.arange(n, dtype=np.int64))
    # hub coefficients in device stripe layout [1, Q, 128]: [0, q, p] = c of
    # the node at rank 128q+p
    cg = np.zeros((1, n), np.float32)
    cg[0] = hub_c[node_of_rank]
    g.hub_c = cg.astype(F16)
    g.hub_pos = int(g.pos_of_node[n - 1])

    groups = []
    qs = 0
    while qs < Q:
        qe = qs + 1
        while qe < Q and g.slot_off[qe + 1] - g.slot_off[qs] <= max_slots:
            qe += 1
        groups.append((qs, qe, int(g.slot_off[qs]), int(g.slot_off[qe])))
        qs = qe
    g.groups = groups
    return g


def _build_resort_idx(g64, n):
    i = np.arange(n)
    t = (i % 128) * g64.Q + (i // 128)
    return g64.node_at_pos[t]


def _build_pool_idx(g_src, g_dst, n_dst):
    idx = np.zeros(4 * n_dst, np.int64)
    Qd = g_dst.Q
    for rr in range(n_dst):
        q, p = rr // 128, rr % 128
        m = g_dst.node_at_pos[p * Qd + q]
        for j in range(4):
            idx[(q * 4 + j) * 128 + p] = g_src.pos_of_node[4 * m + j]
    return idx


def _fold_bn(Wc, bc, gamma, beta):
    s = (gamma / np.sqrt(1.0 + EPS)).astype(np.float32)
    return (Wc * s[None, None, :]).astype(np.float32), (bc * s + beta).astype(np.float32)


def _block_einsum_pieces(Wp, bp, cin, cout):
    K = Wp.shape[0]
    pieces = []
    if cin == 1:
        l_k = []
        for k in range(K):
            l = np.zeros((128, 4 * cout), np.float32)
            for b in range(4):
                l[b, b * cout:(b + 1) * cout] = Wp[k, 0]
            l_k.append(l)
        pieces.append(dict(mats=l_k, e=0, bias=np.tile(bp, 4),
                           segs=[(0, 4 * cout, 0)]))
    elif cin == 32:
        if cout <= 32:
            l_k = []
            for k in range(K):
                l = np.zeros((128, 4 * cout), np.float32)
                for b in range(4):
                    l[b * 32:(b + 1) * 32, b * cout:(b + 1) * cout] = Wp[k]
                l_k.append(l)
            pieces.append(dict(mats=l_k, e=0, bias=np.tile(bp, 4),
                               segs=[(0, 4 * cout, 0)]))
        else:
            for piece in range(2):
                l_k = []
                for k in range(K):
                    l = np.zeros((128, 2 * cout), np.float32)
                    for bi in range(2):
                        b = piece * 2 + bi
                        l[b * 32:(b + 1) * 32, bi * cout:(bi + 1) * cout] = Wp[k]
                    l_k.append(l)
                pieces.append(dict(mats=l_k, e=0, bias=np.tile(bp, 2),
                                   segs=[(0, 2 * cout, piece * 2 * cout)]))
    elif cin == 64:
        if cout <= 64:
            for e in range(2):
                l_k = []
                for k in range(K):
                    l = np.zeros((128, 2 * cout), np.float32)
                    for bi in range(2):
                        l[bi * 64:(bi + 1) * 64, bi * cout:(bi + 1) * cout] = Wp[k]
                    l_k.append(l)
                pieces.append(dict(mats=l_k, e=e, bias=np.tile(bp, 2),
                                   segs=[(0, cout, (2 * e) * cout),
                                         (cout, cout, (2 * e + 1) * cout)]))
        else:
            for e in range(2):
                for h in range(2):
                    l_k = []
                    for k in range(K):
                        l = np.zeros((128, 128), np.float32)
                        for bi in range(2):
                            l[bi * 64:(bi + 1) * 64, bi * 64:(bi + 1) * 64] = \
                                Wp[k][:, h * 64:(h + 1) * 64]
                        l_k.append(l)
                    pieces.append(dict(
                        mats=l_k, e=e,
                        bias=np.tile(bp[h * 64:(h + 1) * 64], 2),
                        segs=[(0, 64, (2 * e) * cout + h * 64),
                              (64, 64, (2 * e + 1) * cout + h * 64)]))
    elif cin == 128:
        nh = (cout + 127) // 128
        for e in range(4):
            for h in range(nh):
                l_k = [np.ascontiguousarray(Wp[k][:, h * 128:(h + 1) * 128])
                       for k in range(K)]
                M = l_k[0].shape[1]
                pieces.append(dict(mats=l_k, e=e,
                                   bias=bp[h * 128:h * 128 + M].copy(),
                                   segs=[(0, M, e * cout + h * 128)]))
    else:
        raise ValueError(cin)
    return pieces


CHANS = [(1, 32), (32, 32), (32, 64), (64, 64), (64, 128), (128, 128),
         (128, 256)]


def _build_structs(inp):
    st = {}
    st["g64"] = _build_graph_struct(inp["src64"], inp["dst64"], inp["w64"],
                                    12288, 48)
    st["g32"] = _build_graph_struct(inp["src32"], inp["dst32"], inp["w32"],
                                    3072, 24)
    st["g16"] = _build_graph_struct(inp["src16"], inp["dst16"], inp["w16"],
                                    768, 12)
    st["idx_resort"] = _wrap_idx_tile(_build_resort_idx(st["g64"], 12288))
    st["idx_pool32"] = _wrap_idx_tile(_build_pool_idx(st["g64"], st["g32"], 3072))
    st["idx_pool16"] = _wrap_idx_tile(_build_pool_idx(st["g32"], st["g16"], 768))
    st["pieces"] = []
    for i, (ci, co) in enumerate(CHANS):
        Wp, bp = _fold_bn(inp[f"Wc{i}"], inp[f"bc{i}"], inp[f"g{i}"], inp[f"b{i}"])
        st["pieces"].append(_block_einsum_pieces(Wp, bp, ci, co))
    return st


# ----------------------------------------------------------------------
# Device program
# ----------------------------------------------------------------------

def _build_bass(st, upto=99, dump=None):
    from concourse import bacc, mybir
    import concourse.tile as tile
    from concourse.masks import make_identity

    f16 = mybir.dt.float16
    f32 = mybir.dt.float32
    i16 = mybir.dt.int16

    nc = bacc.Bacc("TRN2", target_bir_lowering=False, debug=False)
    dev_in = {}

    def din(name, arr):
        arr = np.ascontiguousarray(arr)
        dt = {np.dtype(np.float32): f32, np.dtype(np.float16): f16,
              np.dtype(np.int16): i16}[arr.dtype]
        h = nc.dram_tensor(name, list(arr.shape), dt, kind="ExternalInput").ap()
        dev_in[name] = arr
        return h

    xin = nc.dram_tensor("xin", [4, 12288], f32, kind="ExternalInput").ap()
    dev_in["xin"] = None  # filled per-core
    out_d = nc.dram_tensor("out", [1, 4], f32, kind="ExternalOutput").ap()

    gs = {64: st["g64"], 32: st["g32"], 16: st["g16"]}
    # graph tensors
    gh = {}
    for s, g in gs.items():
        gh[s] = dict(
            idxp=din(f"idxp{s}", g.idx_prop_tile),
            wl=din(f"wl{s}", g.wl),
            wl2=din(f"wl2{s}", g.wl2),
            idxe=din(f"idxe{s}", g.idx_eins_tile),
            hubc=din(f"hubc{s}", g.hub_c),
        )
    idxr = din("idxr", st["idx_resort"])
    idxpool = {32: din("idxpool32", st["idx_pool32"]),
               16: din("idxpool16", st["idx_pool16"])}

    # per-block einsum mats / biases
    mats_h, bias_h, Ms = [], [], []
    for i, pieces in enumerate(st["pieces"]):
        M = pieces[0]["mats"][0].shape[1]
        npc = len(pieces)
        mats = np.zeros((npc * 6, 128, M), F16)
        bias = np.zeros((npc, 128, 1), np.float32)
        for pi, pc in enumerate(pieces):
            for k in range(6):
                mats[pi * 6 + k] = pc["mats"][k].astype(F16)
            bias[pi, :pc["bias"].shape[0], 0] = pc["bias"]
        mats_h.append(din(f"mats{i}", mats))
        bias_h.append(din(f"bias{i}", bias))
        Ms.append(M)

    # head weights
    Wf1 = (np.asarray(st["Wf1"]) / 768.0).astype(np.float32)
    wf1 = np.zeros((2, 2, 128, 128), np.float32)
    for a in range(2):
        for b in range(2):
            wf1[a, b] = Wf1[a * 128:(a + 1) * 128, b * 128:(b + 1) * 128]
    wf1_h = din("wf1", wf1)
    bf1 = np.zeros((2, 128, 1), np.float32)
    bf1[:, :, 0] = np.asarray(st["bf1"]).reshape(2, 128)
    bf1_h = din("bf1r", bf1)
    wf2 = np.zeros((2, 128, 128), np.float32)
    for a in range(2):
        wf2[a] = np.asarray(st["Wf2"])[a * 128:(a + 1) * 128, :]
    wf2_h = din("wf2", wf2)
    bf2 = np.zeros((128, 1), np.float32)
    bf2[:, 0] = np.asarray(st["bf2"])
    bf2_h = din("bf2r", bf2)
    wo_h = din("wo", np.asarray(st["Wo"]).astype(np.float32))
    bo_h = din("bor", np.asarray(st["bo"]).reshape(1, 1).astype(np.float32))

    # DRAM scratch tables
    def dscr(name, rows, erow):
        return nc.dram_tensor(name, [rows, erow], f16).ap()

    xorig = dscr("xorig", 12288, 128)
    NS = [64, 64, 64, 32, 32, 16, 16]
    NN = {64: 12288, 32: 3072, 16: 768}
    EIN = [128, 128, 128, 256, 256, 512, 512]
    EOUT = [128, 128, 256, 256, 512, 512, None]
    tabs = []
    outs_t = []
    for i in range(7):
        tabs.append([dscr(f"T{i}_{k}", NN[NS[i]], EIN[i]) for k in range(1, 6)])
        outs_t.append(dscr(f"bout{i}", NN[NS[i]], EOUT[i]) if EOUT[i] else None)
    pool_t = {32: dscr("pool32", 3072, 256), 16: dscr("pool16", 768, 512)}

    AF = mybir.ActivationFunctionType
    ALU = mybir.AluOpType

    with tile.TileContext(nc) as tc:
        from contextlib import ExitStack
        with ExitStack() as top:
            constp = top.enter_context(tc.tile_pool(name="const", bufs=1))
            ident = constp.tile([128, 128], f16)
            make_identity(nc, ident[:])
            acc0 = constp.tile([128, 4, 2], f32)
            acc1 = constp.tile([128, 4, 2], f32)
            nc.vector.memset(acc0[:], 0.0)
            nc.vector.memset(acc1[:], 0.0)

            def conv_block(ph, blk, g, s, in_table, cin, cout, last=False):
                CE = 4 * cin
                ER = EIN[blk]
                Q = g.Q
                with ExitStack() as ctx:
                    stgp = ctx.enter_context(
                        tc.tile_pool(name=f"stg{blk}", bufs=1))
                    stg = [stgp.tile([128, Q, ER], f16, tag=f"stg{blk}_{j}",
                                     name=f"stg{blk}_{j}") for j in range(3)]
                    if cin == 1:
                        for t in stg:
                            nc.vector.memset(t[:], 0.0)
                    # stg[0] <- in_table (k=2 subtract source)
                    nc.sync.dma_start(
                        out=stg[0][:],
                        in_=in_table.rearrange("(p q) e -> p q e", p=128))
                    gp = ctx.enter_context(tc.tile_pool(name=f"g{blk}", bufs=3))
                    mp = ctx.enter_context(tc.tile_pool(name=f"m{blk}", bufs=3))
                    pp = ctx.enter_context(
                        tc.tile_pool(name=f"ps{blk}", bufs=6, space="PSUM"))
                    idxt = ph["idxp_t"]
                    hp_ = ctx.enter_context(tc.tile_pool(name=f"hub{blk}", bufs=3))
                    for k in range(1, 6):
                        src = in_table if k == 1 else tabs[blk][k - 2]
                        wlt = ph["wl_t"] if k == 1 else ph["wl2_t"]
                        hubt = ph["hubc_t"]
                        hubrow = hp_.tile([1, ER], f16, tag="hubrow",
                                          name="hubrow")
                        nc.sync.dma_start(
                            out=hubrow[:],
                            in_=src[ph["hub_pos"]:ph["hub_pos"] + 1, :])
                        if k > 1:
                            hubrow2 = hp_.tile([1, ER], f16, tag="hubrow2",
                                               name="hubrow2")
                            nc.vector.tensor_scalar_mul(hubrow2[:],
                                                        hubrow[:], 2.0)
                            hubrow = hubrow2
                        cur = stg[k % 3]
                        prev2 = stg[(k - 2) % 3]
                        for (qs, qe, ss, se) in g.groups:
                            ns = se - ss
                            gout = gp.tile([128, ns, ER], f16, tag="gout")
                            nc.gpsimd.dma_gather(
                                out_ap=gout[:], in_ap=src[:],
                                idxs_ap=idxt[:, ss * 8:se * 8],
                                num_idxs=ns * 128, num_idxs_reg=ns * 128,
                                elem_size=ER, single_packet=False)
                            mout = mp.tile([128, ns, CE], f16, tag="mout")
                            nc.vector.tensor_tensor(
                                out=mout[:], in0=gout[:, :, 0:CE],
                                in1=wlt[:, ss:se].broadcast_to([128, ns, CE]),
                                op=ALU.mult)
                            for q in range(qs, qe):
                                s0 = int(g.slot_off[q]) - ss
                                pd = int(g.pads[q])
                                ps = pp.tile([128, CE], f32, tag="red")
                                nc.tensor.matmul(
                                    out=ps[:],
                                    lhsT=hubt[0:1, q * 128:(q + 1) * 128],
                                    rhs=hubrow[0:1, 0:CE],
                                    start=True, stop=False)
                                for j in range(pd):
                                    nc.tensor.matmul(
                                        out=ps[:], lhsT=ident[:, 0:128],
                                        rhs=mout[:, s0 + j, :],
                                        start=False, stop=(j == pd - 1))
                                if k == 1:
                                    nc.vector.tensor_copy(
                                        out=cur[:, q, 0:CE], in_=ps[:])
                                else:
                                    nc.vector.tensor_tensor(
                                        out=cur[:, q, 0:CE], in0=ps[:],
                                        in1=prev2[:, q, 0:CE],
                                        op=ALU.subtract)
                        nc.sync.dma_start(
                            out=tabs[blk][k - 1].rearrange(
                                "(p q) e -> p q e", p=128),
                            in_=cur[:])
                # einsum pass
                n = NN[s]
                pieces = st["pieces"][blk]
                M = Ms[blk]
                e128 = ER // 128
                with ExitStack() as ctx:
                    wp = ctx.enter_context(tc.tile_pool(name=f"w{blk}", bufs=1))
                    matsb = wp.tile([128, len(pieces) * 6, M], f16)
                    nc.sync.dma_start(
                        out=matsb[:],
                        in_=mats_h[blk].rearrange("j r m -> r j m"))
                    biasb = wp.tile([128, len(pieces)], f32)
                    nc.sync.dma_start(
                        out=biasb[:],
                        in_=bias_h[blk].rearrange("j r one -> r (j one)"))
                    tp = ctx.enter_context(tc.tile_pool(name=f"tg{blk}", bufs=2))
                    ep = ctx.enter_context(
                        tc.tile_pool(name=f"eps{blk}", bufs=4, space="PSUM"))
                    op_ = ctx.enter_context(tc.tile_pool(name=f"o{blk}", bufs=3))
                    tpp = ctx.enter_context(
                        tc.tile_pool(name=f"tps{blk}", bufs=2, space="PSUM"))
                    pb = ctx.enter_context(tc.tile_pool(name=f"pb{blk}", bufs=3))
                    sgw = {64: 2048, 32: 1024, 16: 768}[s]
                    ktabs = [in_table] + tabs[blk]
                    for c0 in range(0, n, sgw):
                        cw = min(sgw, n - c0)
                        tg = []
                        for k in range(6):
                            t = tp.tile([128, e128, cw], f16, tag=f"tg{k}", name=f"tg{k}")
                            for e in range(e128):
                                nc.sync.dma_start(
                                    out=t[:, e, :],
                                    in_=ktabs[k][c0:c0 + cw,
                                                 e * 128:(e + 1) * 128],
                                    transpose=True)
                            tg.append(t)
                        for pi, pc in enumerate(pieces):
                            for cc in range(0, cw, 512):
                                cl = min(512, cw - cc)
                                ps = ep.tile([128, 512], f32, tag="eps")
                                for k in range(6):
                                    nc.tensor.matmul(
                                        out=ps[:, 0:cl],
                                        lhsT=matsb[:, pi * 6 + k, :],
                                        rhs=tg[k][:, pc["e"], cc:cc + cl],
                                        start=(k == 0), stop=(k == 5))
                                if last:
                                    scrap = op_.tile([128, 512], f16, tag="scrap")
                                    accs = acc0 if (pi % 2 == 0) else acc1
                                    b = pc["e"]
                                    nc.scalar.activation(
                                        out=scrap[:, 0:cl], in_=ps[:, 0:cl],
                                        func=AF.Relu,
                                        bias=biasb[:, pi:pi + 1],
                                        accum_out=accs[:, b:b + 1,
                                                       cc // 512:cc // 512 + 1])
                                else:
                                    o16 = op_.tile([128, 512], f16, tag="o16")
                                    nc.scalar.activation(
                                        out=o16[:, 0:cl], in_=ps[:, 0:cl],
                                        func=AF.Relu,
                                        bias=biasb[:, pi:pi + 1])
                                    for c4 in range(0, cl, 128):
                                        pst = tpp.tile([128, 128], f16, tag="pst")
                                        nc.tensor.transpose(
                                            out=pst[:],
                                            in_=o16[:, c4:c4 + 128],
                                            identity=ident[:])
                                        pcs = pb.tile([128, 128], f16, tag="pcs")
                                        nc.vector.tensor_copy(out=pcs[:], in_=pst[:])
                                        r0 = c0 + cc + c4
                                        for (sc, w_, roff) in pc["segs"]:
                                            nc.sync.dma_start(
                                                out=outs_t[blk][r0:r0 + 128,
                                                                roff:roff + w_],
                                                in_=pcs[:, sc:sc + w_])

            def pool_block(ph, s_dst, in_table, er_src):
                g = gs[s_dst]
                Qd = g.Q
                n_dst = NN[s_dst]
                with ExitStack() as ctx:
                    gp = ctx.enter_context(tc.tile_pool(name=f"plg{s_dst}", bufs=3))
                    pp = ctx.enter_context(
                        tc.tile_pool(name=f"plp{s_dst}", bufs=4, space="PSUM"))
                    stp = ctx.enter_context(tc.tile_pool(name=f"pls{s_dst}", bufs=1))
                    stg = stp.tile([128, Qd, er_src], f16)
                    grp = {32: 6, 16: 3}[s_dst]  # stripes per gather
                    for q0 in range(0, Qd, grp):
                        qn = min(grp, Qd - q0)
                        ns = qn * 4
                        gout = gp.tile([128, ns, er_src], f16, tag="plgout")
                        nc.gpsimd.dma_gather(
                            out_ap=gout[:], in_ap=in_table[:],
                            idxs_ap=idxpool_t[s_dst][:, q0 * 4 * 8:(q0 + qn) * 4 * 8],
                            num_idxs=ns * 128, num_idxs_reg=ns * 128,
                            elem_size=er_src, single_packet=False)
                        for qq in range(qn):
                            ps = pp.tile([128, er_src], f32, tag="plps")
                            for j in range(4):
                                nc.tensor.matmul(
                                    out=ps[:], lhsT=ident[:, 0:128],
                                    rhs=gout[:, qq * 4 + j, :],
                                    start=(j == 0), stop=(j == 3))
                            nc.vector.tensor_scalar_mul(stg[:, q0 + qq, :],
                                                        ps[:], 0.25)
                    nc.sync.dma_start(
                        out=pool_t[s_dst].rearrange("(p q) e -> p q e", p=128),
                        in_=stg[:])

            # ---------------- emit ----------------
            phases = {}
            for s in (64, 32, 16):
                g = gs[s]
                ph = phases[s] = {}
                ph["idxp_t"] = constp.tile([128, g.S * 8], i16, name=f"idxpt{s}")
                nc.sync.dma_start(out=ph["idxp_t"][:], in_=gh[s]["idxp"][:])
                ph["wl_t"] = constp.tile([128, g.S], f16, name=f"wlt{s}")
                nc.sync.dma_start(out=ph["wl_t"][:], in_=gh[s]["wl"][:])
                ph["wl2_t"] = constp.tile([128, g.S], f16, name=f"wl2t{s}")
                nc.sync.dma_start(out=ph["wl2_t"][:], in_=gh[s]["wl2"][:])
                ph["idxe_t"] = constp.tile([128, gs[s].n // 16], i16, name=f"idxet{s}")
                nc.sync.dma_start(out=ph["idxe_t"][:], in_=gh[s]["idxe"][:])
                ph["hubc_t"] = constp.tile([1, gs[s].n], f16, name=f"hubct{s}")
                nc.sync.dma_start(out=ph["hubc_t"][:], in_=gh[s]["hubc"][:])
                ph["hub_pos"] = gs[s].hub_pos
            idxpool_t = {}
            for s in (32, 16):
                idxpool_t[s] = constp.tile([128, gs[s].n // 4], i16,
                                           name=f"idxpoolt{s}")
                nc.sync.dma_start(out=idxpool_t[s][:], in_=idxpool[s][:])

            # x prep: build xorig then resort into block0 input staging
            with ExitStack() as ctx:
                xp = ctx.enter_context(tc.tile_pool(name="xp", bufs=1))
                xb = xp.tile([128, 96, 128], f16)
                nc.vector.memset(xb[:], 0.0)
                for b in range(4):
                    xf = xp.tile([128, 96], f32, tag=f"xf{b}")
                    nc.sync.dma_start(
                        out=xf[:],
                        in_=xin[b, :].rearrange("(p q) -> p q", p=128))
                    nc.vector.tensor_copy(out=xb[:, :, b], in_=xf[:])
                nc.sync.dma_start(
                    out=xorig.rearrange("(p q) e -> p q e", p=128), in_=xb[:])
                irt = xp.tile([128, 768], i16)
                nc.sync.dma_start(out=irt[:], in_=idxr[:])
                x0 = xp.tile([128, 96, 128], f16)
                nc.gpsimd.dma_gather(
                    out_ap=x0[:], in_ap=xorig[:], idxs_ap=irt[:],
                    num_idxs=12288, num_idxs_reg=12288, elem_size=128,
                    single_packet=False)
                # block0 input table
                b0in = nc.dram_tensor("b0in", [12288, 128], f16).ap()
                nc.sync.dma_start(
                    out=b0in.rearrange("(p q) e -> p q e", p=128), in_=x0[:])

            def _emit_head():
                with ExitStack() as ctx:
                    hp = ctx.enter_context(tc.tile_pool(name="head", bufs=1))
                    hpp = ctx.enter_context(
                        tc.tile_pool(name="headps", bufs=2, space="PSUM"))
                    w1 = hp.tile([128, 2, 2, 128], f32)
                    nc.sync.dma_start(out=w1[:], in_=wf1_h.rearrange("a b r m -> r a b m"))
                    b1 = hp.tile([128, 2], f32)
                    nc.sync.dma_start(out=b1[:], in_=bf1_h.rearrange("j r one -> r (j one)"))
                    w2 = hp.tile([128, 2, 128], f32)
                    nc.sync.dma_start(out=w2[:], in_=wf2_h.rearrange("a r m -> r a m"))
                    b2 = hp.tile([128, 1], f32)
                    nc.sync.dma_start(out=b2[:], in_=bf2_h[:])
                    wo = hp.tile([128, 1], f32)
                    nc.sync.dma_start(out=wo[:], in_=wo_h[:])
                    bo = hp.tile([1, 1], f32)
                    nc.sync.dma_start(out=bo[:], in_=bo_h[:])
                    accsum = [hp.tile([128, 4], f32, tag=f"as{a}",
                                      name=f"accsum{a}") for a in range(2)]
                    for a, acct in enumerate((acc0, acc1)):
                        nc.vector.tensor_add(out=accsum[a][:],
                                             in0=acct[:, :, 0],
                                             in1=acct[:, :, 1])
                    accs = accsum
                    z1 = [hp.tile([128, 4], f32, tag=f"z1_{b}", name=f"z1_{b}")
                          for b in range(2)]
                    for bb in range(2):
                        ps = hpp.tile([128, 4], f32, tag="hps")
                        for a in range(2):
                            nc.tensor.matmul(out=ps[:], lhsT=w1[:, a, bb, :],
                                             rhs=accs[a][:], start=(a == 0),
                                             stop=(a == 1))
                        nc.scalar.activation(out=z1[bb][:], in_=ps[:], func=AF.Relu,
                                             bias=b1[:, bb:bb + 1])
                    ps2 = hpp.tile([128, 4], f32, tag="hps2")
                    for a in range(2):
                        nc.tensor.matmul(out=ps2[:], lhsT=w2[:, a, :], rhs=z1[a][:],
                                         start=(a == 0), stop=(a == 1))
                    z2 = hp.tile([128, 4], f32)
                    nc.scalar.activation(out=z2[:], in_=ps2[:], func=AF.Relu,
                                         bias=b2[:, 0:1])
                    ps3 = hpp.tile([1, 4], f32, tag="hps3")
                    nc.tensor.matmul(out=ps3[:], lhsT=wo[:], rhs=z2[:],
                                     start=True, stop=True)
                    osb = hp.tile([1, 4], f32)
                    nc.scalar.activation(out=osb[:], in_=ps3[:], func=AF.Identity,
                                         bias=bo[:])
                    nc.sync.dma_start(out=out_d[:], in_=osb[:])

            stages = [
                lambda: conv_block(phases[64], 0, gs[64], 64, b0in, 1, 32),
                lambda: conv_block(phases[64], 1, gs[64], 64, outs_t[0], 32, 32),
                lambda: conv_block(phases[64], 2, gs[64], 64, outs_t[1], 32, 64),
                lambda: pool_block(phases[64], 32, outs_t[2], 256),
                lambda: conv_block(phases[32], 3, gs[32], 32, pool_t[32], 64, 64),
                lambda: conv_block(phases[32], 4, gs[32], 32, outs_t[3], 64, 128),
                lambda: pool_block(phases[32], 16, outs_t[4], 512),
                lambda: conv_block(phases[16], 5, gs[16], 16, pool_t[16], 128, 128),
                lambda: conv_block(phases[16], 6, gs[16], 16, outs_t[5], 128,
                                   256, last=True),
            ]
            for si, fn in enumerate(stages):
                if si < upto:
                    fn()
            if dump is not None:
                dt_map = {"b0in": b0in, "out0": outs_t[0], "out1": outs_t[1],
                          "out2": outs_t[2], "pool32": pool_t[32],
                          "out3": outs_t[3], "out4": outs_t[4],
                          "pool16": pool_t[16], "out5": outs_t[5],
                          "T0_1": tabs[0][0], "T1_1": tabs[1][0],
                          "T0_5": tabs[0][4]}
                src = dt_map[dump]
                rows, erow = src.shape
                dbg = nc.dram_tensor("dbg", [rows, erow], f16,
                                     kind="ExternalOutput").ap()
                with tc.tile_pool(name="dbgp", bufs=2) as dp:
                    for r0 in range(0, rows, 128):
                        t = dp.tile([128, erow], f16, tag="dbgt")
                        nc.sync.dma_start(out=t[:], in_=src[r0:r0 + 128, :])
                        nc.sync.dma_start(out=dbg[r0:r0 + 128, :], in_=t[:])

            if upto < 9:
                with tc.tile_pool(name="zo", bufs=1) as zp:
                    zt = zp.tile([1, 4], f32)
                    nc.vector.memset(zt[:], 0.0)
                    nc.sync.dma_start(out=out_d[:], in_=zt[:])

            # ---------------- head ----------------
            if upto >= 9:
                _emit_head()
    nc.compile()
    return nc, dev_in


_CACHE = {}


def kernel(**inputs):
    from concourse.bass_utils import run_bass_kernel_spmd

    inp = {k: np.asarray(v) for k, v in inputs.items()}
    st = _build_structs(inp)
    for k in ("Wf1", "bf1", "Wf2", "bf2", "Wo", "bo"):
        st[k] = inp[k]
    nc, dev_in = _build_bass(st)

    x = inp["x"].reshape(32, 12288).astype(np.float32)
    in_maps = []
    for c in range(8):
        m = dict(dev_in)
        m["xin"] = np.ascontiguousarray(x[4 * c:4 * (c + 1)])
        in_maps.append(m)
    res = run_bass_kernel_spmd(nc, in_maps, list(range(8)))
    out = np.zeros((32, 1), np.float32)
    for c in range(8):
        out[4 * c:4 * (c + 1), 0] = res.results[c]["out"][0]
    return out



# revision 11
# speedup vs baseline: 1.1120x; 1.1120x over previous
# Trainium2 Bass kernel for nn_DeepSphere (ChebConv GNN, K=6, 7 blocks + FC).
#
# Strategy: pure batch data-parallel over 8 NeuronCores (4 batch each).
# Per core, each ChebConv propagate is a padded-CSR gather (dma_gather,
# degree-sorted node permutation so per-stripe pads are tight), a DVE
# multiply by the Laplacian edge weights, and a PE identity-matmul
# segmented reduction accumulating in PSUM. The per-order channel mixing
# runs as an end-of-block pass: bf/fp16 transpose-gathers put (b, ci) on
# partitions and host-built block-diagonal weight matrices contract all
# batch elements in single matmuls. Activations live in HBM as fp16
# node-major tables; all accumulation is fp32 (PSUM).
import sys
import numpy as np

sys.path.insert(0, "/opt/trn_rl_repo")

EPS = 1e-5
F16 = np.float16

# ----------------------------------------------------------------------
# Host-side graph preprocessing
# ----------------------------------------------------------------------


def _effective_graph(src, dst, w, n):
    """Replicate JAX OOB semantics: segment_sum/scatter drop idx >= n,
    gather clamps. Returns (src_eff, dst, wl) for the kept edges."""
    src = np.asarray(src).astype(np.int64)
    dst = np.asarray(dst).astype(np.int64)
    w = np.asarray(w).astype(np.float32)
    deg = np.zeros(n, np.float32)
    m = src < n
    np.add.at(deg, src[m], w[m])
    dinv = np.where(deg > 0, 1.0 / np.sqrt(deg), 0.0).astype(np.float32)
    srcc = np.minimum(src, n - 1)
    dstc = np.minimum(dst, n - 1)
    wl = (-w * dinv[srcc] * dinv[dstc]).astype(np.float32)
    keep = dst < n
    srcc, dst, wl = srcc[keep], dst[keep], wl[keep]
    # edges whose (clamped) source is the hub row n-1 collapse to a rank-1
    # term out[n] += c_n * T[n-1]
    hub = srcc == (n - 1)
    c = np.zeros(n, np.float32)
    np.add.at(c, dst[hub], wl[hub])
    return srcc[~hub], dst[~hub], wl[~hub], c


def _wrap_idx_tile(idx):
    idx = np.asarray(idx)
    count = idx.shape[0]
    assert count % 16 == 0
    t = np.zeros((128, count // 16), np.int16)
    base = idx.reshape(count // 16, 16).T.astype(np.int16)
    for g in range(8):
        t[g * 16:(g + 1) * 16, :] = base
    return t


class _G:
    pass


def _build_graph_struct(src, dst, w, n, max_slots):
    g = _G()
    g.n = n
    Q = n // 128
    g.Q = Q
    src, dst, wl, hub_c = _effective_graph(src, dst, w, n)
    indeg = np.bincount(dst, minlength=n)
    node_of_rank = np.argsort(-indeg, kind="stable")
    r = np.arange(n)
    pos_of_rank = (r % 128) * Q + (r // 128)
    g.node_at_pos = np.empty(n, np.int64)
    g.node_at_pos[pos_of_rank] = node_of_rank
    g.pos_of_node = np.empty(n, np.int64)
    g.pos_of_node[g.node_at_pos] = np.arange(n)

    deg_by_rank = indeg[node_of_rank]
    # degree-sorted: all-zero stripes cluster at the tail; give them zero
    # slots (no gather descriptors) instead of one padded slot each
    pads = deg_by_rank.reshape(Q, 128).max(axis=1).astype(np.int64)
    g.pads = pads
    g.slot_off = np.concatenate([[0], np.cumsum(pads)])
    g.S = int(g.slot_off[-1])

    order = np.argsort(dst, kind="stable")
    dst_sorted = dst[order]
    starts = np.searchsorted(dst_sorted, np.arange(n))
    ends = np.searchsorted(dst_sorted, np.arange(n) + 1)

    idx = np.zeros(g.S * 128, np.int64)
    wlg = np.zeros((128, g.S), np.float32)
    pos_src = g.pos_of_node[src]
    for rr in range(n):
        q, p = rr // 128, rr % 128
        node = node_of_rank[rr]
        es = order[starts[node]:ends[node]]
        s0 = g.slot_off[q]
        idx[(s0 + np.arange(len(es))) * 128 + p] = pos_src[es]
        wlg[p, s0:s0 + len(es)] = wl[es]
    g.idx_prop_tile = _wrap_idx_tile(idx)
    g.wl = wlg.astype(F16)
    g.wl2 = (2.0 * wlg).astype(F16)
    g.idx_eins_tile = _wrap_idx_tile(np.arange(n, dtype=np.int64))
    # hub coefficients in device stripe layout [1, Q, 128]: [0, q, p] = c of
    # the node at rank 128q+p
    cg = np.zeros((1, n), np.float32)
    cg[0] = hub_c[node_of_rank]
    g.hub_c = cg.astype(F16)
    g.hub_pos = int(g.pos_of_node[n - 1])

    groups = []
    qs = 0
    while qs < Q:
        qe = qs + 1
        while qe < Q and g.slot_off[qe + 1] - g.slot_off[qs] <= max_slots:
            qe += 1
        groups.append((qs, qe, int(g.slot_off[qs]), int(g.slot_off[qe])))
        qs = qe
    g.groups = groups
    return g


def _build_resort_idx(g64, n):
    i = np.arange(n)
    t = (i % 128) * g64.Q + (i // 128)
    return g64.node_at_pos[t]


def _build_pool_idx(g_src, g_dst, n_dst):
    idx = np.zeros(4 * n_dst, np.int64)
    Qd = g_dst.Q
    for rr in range(n_dst):
        q, p = rr // 128, rr % 128
        m = g_dst.node_at_pos[p * Qd + q]
        for j in range(4):
            idx[(q * 4 + j) * 128 + p] = g_src.pos_of_node[4 * m + j]
    return idx


def _fold_bn(Wc, bc, gamma, beta):
    s = (gamma / np.sqrt(1.0 + EPS)).astype(np.float32)
    return (Wc * s[None, None, :]).astype(np.float32), (bc * s + beta).astype(np.float32)


def _block_einsum_pieces(Wp, bp, cin, cout):
    K = Wp.shape[0]
    pieces = []
    if cin == 1:
        l_k = []
        for k in range(K):
            l = np.zeros((128, 4 * cout), np.float32)
            for b in range(4):
                l[b, b * cout:(b + 1) * cout] = Wp[k, 0]
            l_k.append(l)
        pieces.append(dict(mats=l_k, e=0, bias=np.tile(bp, 4),
                           segs=[(0, 4 * cout, 0)]))
    elif cin == 32:
        if cout <= 32:
            l_k = []
            for k in range(K):
                l = np.zeros((128, 4 * cout), np.float32)
                for b in range(4):
                    l[b * 32:(b + 1) * 32, b * cout:(b + 1) * cout] = Wp[k]
                l_k.append(l)
            pieces.append(dict(mats=l_k, e=0, bias=np.tile(bp, 4),
                               segs=[(0, 4 * cout, 0)]))
        else:
            for piece in range(2):
                l_k = []
                for k in range(K):
                    l = np.zeros((128, 2 * cout), np.float32)
                    for bi in range(2):
                        b = piece * 2 + bi
                        l[b * 32:(b + 1) * 32, bi * cout:(bi + 1) * cout] = Wp[k]
                    l_k.append(l)
                pieces.append(dict(mats=l_k, e=0, bias=np.tile(bp, 2),
                                   segs=[(0, 2 * cout, piece * 2 * cout)]))
    elif cin == 64:
        if cout <= 64:
            for e in range(2):
                l_k = []
                for k in range(K):
                    l = np.zeros((128, 2 * cout), np.float32)
                    for bi in range(2):
                        l[bi * 64:(bi + 1) * 64, bi * cout:(bi + 1) * cout] = Wp[k]
                    l_k.append(l)
                pieces.append(dict(mats=l_k, e=e, bias=np.tile(bp, 2),
                                   segs=[(0, cout, (2 * e) * cout),
                                         (cout, cout, (2 * e + 1) * cout)]))
        else:
            for e in range(2):
                for h in range(2):
                    l_k = []
                    for k in range(K):
                        l = np.zeros((128, 128), np.float32)
                        for bi in range(2):
                            l[bi * 64:(bi + 1) * 64, bi * 64:(bi + 1) * 64] = \
                                Wp[k][:, h * 64:(h + 1) * 64]
                        l_k.append(l)
                    pieces.append(dict(
                        mats=l_k, e=e,
                        bias=np.tile(bp[h * 64:(h + 1) * 64], 2),
                        segs=[(0, 64, (2 * e) * cout + h * 64),
                              (64, 64, (2 * e + 1) * cout + h * 64)]))
    elif cin == 128:
        nh = (cout + 127) // 128
        for e in range(4):
            for h in range(nh):
                l_k = [np.ascontiguousarray(Wp[k][:, h * 128:(h + 1) * 128])
                       for k in range(K)]
                M = l_k[0].shape[1]
                pieces.append(dict(mats=l_k, e=e,
                                   bias=bp[h * 128:h * 128 + M].copy(),
                                   segs=[(0, M, e * cout + h * 128)]))
    else:
        raise ValueError(cin)
    return pieces


CHANS = [(1, 32), (32, 32), (32, 64), (64, 64), (64, 128), (128, 128),
         (128, 256)]


def _build_structs(inp):
    st = {}
    st["g64"] = _build_graph_struct(inp["src64"], inp["dst64"], inp["w64"],
                                    12288, 48)
    st["g32"] = _build_graph_struct(inp["src32"], inp["dst32"], inp["w32"],
                                    3072, 24)
    st["g16"] = _build_graph_struct(inp["src16"], inp["dst16"], inp["w16"],
                                    768, 12)
    st["idx_resort"] = _wrap_idx_tile(_build_resort_idx(st["g64"], 12288))
    st["idx_pool32"] = _wrap_idx_tile(_build_pool_idx(st["g64"], st["g32"], 3072))
    st["idx_pool16"] = _wrap_idx_tile(_build_pool_idx(st["g32"], st["g16"], 768))
    st["pieces"] = []
    for i, (ci, co) in enumerate(CHANS):
        Wp, bp = _fold_bn(inp[f"Wc{i}"], inp[f"bc{i}"], inp[f"g{i}"], inp[f"b{i}"])
        st["pieces"].append(_block_einsum_pieces(Wp, bp, ci, co))
    return st


# ----------------------------------------------------------------------
# Device program
# ----------------------------------------------------------------------

def _build_bass(st, upto=99, dump=None):
    from concourse import bacc, mybir
    import concourse.tile as tile
    from concourse.masks import make_identity

    f16 = mybir.dt.float16
    f32 = mybir.dt.float32
    i16 = mybir.dt.int16

    nc = bacc.Bacc("TRN2", target_bir_lowering=False, debug=False)
    dev_in = {}

    def din(name, arr):
        arr = np.ascontiguousarray(arr)
        dt = {np.dtype(np.float32): f32, np.dtype(np.float16): f16,
              np.dtype(np.int16): i16}[arr.dtype]
        h = nc.dram_tensor(name, list(arr.shape), dt, kind="ExternalInput").ap()
        dev_in[name] = arr
        return h

    xin = nc.dram_tensor("xin", [4, 12288], f32, kind="ExternalInput").ap()
    dev_in["xin"] = None  # filled per-core
    out_d = nc.dram_tensor("out", [1, 4], f32, kind="ExternalOutput").ap()

    gs = {64: st["g64"], 32: st["g32"], 16: st["g16"]}
    # graph tensors
    gh = {}
    for s, g in gs.items():
        gh[s] = dict(
            idxp=din(f"idxp{s}", g.idx_prop_tile),
            wl=din(f"wl{s}", g.wl),
            wl2=din(f"wl2{s}", g.wl2),
            idxe=din(f"idxe{s}", g.idx_eins_tile),
            hubc=din(f"hubc{s}", g.hub_c),
        )
    idxr = din("idxr", st["idx_resort"])
    idxpool = {32: din("idxpool32", st["idx_pool32"]),
               16: din("idxpool16", st["idx_pool16"])}

    # per-block einsum mats / biases
    mats_h, bias_h, Ms = [], [], []
    for i, pieces in enumerate(st["pieces"]):
        M = pieces[0]["mats"][0].shape[1]
        npc = len(pieces)
        mats = np.zeros((npc * 6, 128, M), F16)
        bias = np.zeros((npc, 128, 1), np.float32)
        for pi, pc in enumerate(pieces):
            for k in range(6):
                mats[pi * 6 + k] = pc["mats"][k].astype(F16)
            bias[pi, :pc["bias"].shape[0], 0] = pc["bias"]
        mats_h.append(din(f"mats{i}", mats))
        bias_h.append(din(f"bias{i}", bias))
        Ms.append(M)

    # head weights
    Wf1 = (np.asarray(st["Wf1"]) / 768.0).astype(np.float32)
    wf1 = np.zeros((2, 2, 128, 128), np.float32)
    for a in range(2):
        for b in range(2):
            wf1[a, b] = Wf1[a * 128:(a + 1) * 128, b * 128:(b + 1) * 128]
    wf1_h = din("wf1", wf1)
    bf1 = np.zeros((2, 128, 1), np.float32)
    bf1[:, :, 0] = np.asarray(st["bf1"]).reshape(2, 128)
    bf1_h = din("bf1r", bf1)
    wf2 = np.zeros((2, 128, 128), np.float32)
    for a in range(2):
        wf2[a] = np.asarray(st["Wf2"])[a * 128:(a + 1) * 128, :]
    wf2_h = din("wf2", wf2)
    bf2 = np.zeros((128, 1), np.float32)
    bf2[:, 0] = np.asarray(st["bf2"])
    bf2_h = din("bf2r", bf2)
    wo_h = din("wo", np.asarray(st["Wo"]).astype(np.float32))
    bo_h = din("bor", np.asarray(st["bo"]).reshape(1, 1).astype(np.float32))

    # DRAM scratch tables
    def dscr(name, rows, erow):
        return nc.dram_tensor(name, [rows, erow], f16).ap()

    xorig = dscr("xorig", 12288, 128)
    NS = [64, 64, 64, 32, 32, 16, 16]
    NN = {64: 12288, 32: 3072, 16: 768}
    EIN = [128, 128, 128, 256, 256, 512, 512]
    EOUT = [128, 128, 256, 256, 512, 512, None]
    tabs = []
    outs_t = []
    for i in range(7):
        tabs.append([dscr(f"T{i}_{k}", NN[NS[i]], EIN[i]) for k in range(1, 6)])
        outs_t.append(dscr(f"bout{i}", NN[NS[i]], EOUT[i]) if EOUT[i] else None)
    pool_t = {32: dscr("pool32", 3072, 256), 16: dscr("pool16", 768, 512)}

    AF = mybir.ActivationFunctionType
    ALU = mybir.AluOpType

    with tile.TileContext(nc) as tc:
        from contextlib import ExitStack
        with ExitStack() as top:
            constp = top.enter_context(tc.tile_pool(name="const", bufs=1))
            ident = constp.tile([128, 128], f16)
            make_identity(nc, ident[:])
            acc0 = constp.tile([128, 4, 2], f32)
            acc1 = constp.tile([128, 4, 2], f32)
            nc.vector.memset(acc0[:], 0.0)
            nc.vector.memset(acc1[:], 0.0)
            dsem = nc.alloc_semaphore("gdma")

            def conv_block(ph, blk, g, s, in_table, cin, cout, last=False):
                CE = 4 * cin
                ER = EIN[blk]
                Q = g.Q
                with ExitStack() as ctx:
                    stgp = ctx.enter_context(
                        tc.tile_pool(name=f"stg{blk}", bufs=1))
                    stg = [stgp.tile([128, Q, ER], f16, tag=f"stg{blk}_{j}",
                                     name=f"stg{blk}_{j}") for j in range(3)]
                    if cin == 1:
                        for t in stg:
                            nc.vector.memset(t[:], 0.0)
                    # stg[0] <- in_table (k=2 subtract source)
                    nc.sync.dma_start(
                        out=stg[0][:],
                        in_=in_table.rearrange("(p q) e -> p q e", p=128))
                    gp = ctx.enter_context(tc.tile_pool(name=f"g{blk}", bufs=3))
                    mp = ctx.enter_context(tc.tile_pool(name=f"m{blk}", bufs=3))
                    pp = ctx.enter_context(
                        tc.tile_pool(name=f"ps{blk}", bufs=6, space="PSUM"))
                    idxt = ph["idxp_t"]
                    hp_ = ctx.enter_context(tc.tile_pool(name=f"hub{blk}", bufs=3))
                    for k in range(1, 6):
                        src = in_table if k == 1 else tabs[blk][k - 2]
                        wlt = ph["wl_t"] if k == 1 else ph["wl2_t"]
                        hubt = ph["hubc_t"]
                        hubrow = hp_.tile([1, ER], f16, tag="hubrow",
                                          name="hubrow")
                        nc.sync.dma_start(
                            out=hubrow[:],
                            in_=src[ph["hub_pos"]:ph["hub_pos"] + 1, :])
                        if k > 1:
                            hubrow2 = hp_.tile([1, ER], f16, tag="hubrow2",
                                               name="hubrow2")
                            nc.vector.tensor_scalar_mul(hubrow2[:],
                                                        hubrow[:], 2.0)
                            hubrow = hubrow2
                        cur = stg[k % 3]
                        prev2 = stg[(k - 2) % 3]
                        # descriptor-gen for the first NPREP groups is hoisted
                        # (prepare_only has no data dep on the source table;
                        # the trigger carries it), so Q7 desc-gen overlaps the
                        # previous k-step's tail instead of stalling on it.
                        NPREP = 0
                        gouts = []
                        for pi_ in range(NPREP):
                            qs, qe, ss, se = g.groups[pi_]
                            ns = se - ss
                            gout = gp.tile([128, ns, ER], f16, tag="gout")
                            nc.gpsimd.dma_gather(
                                out_ap=gout[:], in_ap=src[:],
                                idxs_ap=idxt[:, ss * 8:se * 8],
                                num_idxs=ns * 128, num_idxs_reg=ns * 128,
                                elem_size=ER, single_packet=False,
                                prepare_only=True)
                            gouts.append(gout)
                        for gi_, (qs, qe, ss, se) in enumerate(g.groups):
                            ns = se - ss
                            if ns > 0:
                                if gi_ < NPREP:
                                    gout = gouts[gi_]
                                    nc.gpsimd.trigger_dma(count=1)
                                else:
                                    gout = gp.tile([128, ns, ER], f16,
                                                   tag="gout")
                                    nc.gpsimd.dma_gather(
                                        out_ap=gout[:], in_ap=src[:],
                                        idxs_ap=idxt[:, ss * 8:se * 8],
                                        num_idxs=ns * 128,
                                        num_idxs_reg=ns * 128,
                                        elem_size=ER, single_packet=False)
                                mout = mp.tile([128, ns, CE], f16, tag="mout")
                                nc.vector.tensor_tensor(
                                    out=mout[:], in0=gout[:, :, 0:CE],
                                    in1=wlt[:, ss:se].broadcast_to(
                                        [128, ns, CE]),
                                    op=ALU.mult)
                            for q in range(qs, qe):
                                s0 = int(g.slot_off[q]) - ss
                                pd = int(g.pads[q])
                                ps = pp.tile([128, CE], f32, tag="red")
                                nc.tensor.matmul(
                                    out=ps[:],
                                    lhsT=hubt[0:1, q * 128:(q + 1) * 128],
                                    rhs=hubrow[0:1, 0:CE],
                                    start=True, stop=(pd == 0))
                                for j in range(pd):
                                    nc.tensor.matmul(
                                        out=ps[:], lhsT=ident[:, 0:128],
                                        rhs=mout[:, s0 + j, :],
                                        start=False, stop=(j == pd - 1))
                                if k == 1:
                                    nc.vector.tensor_copy(
                                        out=cur[:, q, 0:CE], in_=ps[:])
                                else:
                                    nc.vector.tensor_tensor(
                                        out=cur[:, q, 0:CE], in0=ps[:],
                                        in1=prev2[:, q, 0:CE],
                                        op=ALU.subtract)
                            # store this group's stripes immediately so the
                            # next k-step's trigger fires as soon as the last
                            # (smallest) group lands, not after a 3MB store
                            nc.sync.dma_start(
                                out=tabs[blk][k - 1].rearrange(
                                    "(p q) e -> p q e", p=128)[:, qs:qe, :],
                                in_=cur[:, qs:qe, :])
                # einsum pass
                n = NN[s]
                pieces = st["pieces"][blk]
                M = Ms[blk]
                e128 = ER // 128
                with ExitStack() as ctx:
                    wp = ctx.enter_context(tc.tile_pool(name=f"w{blk}", bufs=1))
                    matsb = wp.tile([128, len(pieces) * 6, M], f16)
                    nc.sync.dma_start(
                        out=matsb[:],
                        in_=mats_h[blk].rearrange("j r m -> r j m"))
                    biasb = wp.tile([128, len(pieces)], f32)
                    nc.sync.dma_start(
                        out=biasb[:],
                        in_=bias_h[blk].rearrange("j r one -> r (j one)"))
                    tp = ctx.enter_context(tc.tile_pool(name=f"tg{blk}", bufs=2))
                    ep = ctx.enter_context(
                        tc.tile_pool(name=f"eps{blk}", bufs=4, space="PSUM"))
                    op_ = ctx.enter_context(tc.tile_pool(name=f"o{blk}", bufs=3))
                    tpp = ctx.enter_context(
                        tc.tile_pool(name=f"tps{blk}", bufs=2, space="PSUM"))
                    pb = ctx.enter_context(tc.tile_pool(name=f"pb{blk}", bufs=3))
                    sgw = {64: 2048, 32: 1024, 16: 768}[s]
                    ktabs = [in_table] + tabs[blk]
                    for c0 in range(0, n, sgw):
                        cw = min(sgw, n - c0)
                        tg = []
                        for k in range(6):
                            t = tp.tile([128, e128, cw], f16, tag=f"tg{k}", name=f"tg{k}")
                            for e in range(e128):
                                nc.sync.dma_start(
                                    out=t[:, e, :],
                                    in_=ktabs[k][c0:c0 + cw,
                                                 e * 128:(e + 1) * 128],
                                    transpose=True)
                            tg.append(t)
                        for pi, pc in enumerate(pieces):
                            for cc in range(0, cw, 512):
                                cl = min(512, cw - cc)
                                ps = ep.tile([128, 512], f32, tag="eps")
                                for k in range(6):
                                    nc.tensor.matmul(
                                        out=ps[:, 0:cl],
                                        lhsT=matsb[:, pi * 6 + k, :],
                                        rhs=tg[k][:, pc["e"], cc:cc + cl],
                                        start=(k == 0), stop=(k == 5))
                                if last:
                                    scrap = op_.tile([128, 512], f16, tag="scrap")
                                    accs = acc0 if (pi % 2 == 0) else acc1
                                    b = pc["e"]
                                    nc.scalar.activation(
                                        out=scrap[:, 0:cl], in_=ps[:, 0:cl],
                                        func=AF.Relu,
                                        bias=biasb[:, pi:pi + 1],
                                        accum_out=accs[:, b:b + 1,
                                                       cc // 512:cc // 512 + 1])
                                else:
                                    o16 = op_.tile([128, 512], f16, tag="o16")
                                    nc.scalar.activation(
                                        out=o16[:, 0:cl], in_=ps[:, 0:cl],
                                        func=AF.Relu,
                                        bias=biasb[:, pi:pi + 1])
                                    for c4 in range(0, cl, 128):
                                        pst = tpp.tile([128, 128], f16, tag="pst")
                                        nc.tensor.transpose(
                                            out=pst[:],
                                            in_=o16[:, c4:c4 + 128],
                                            identity=ident[:])
                                        pcs = pb.tile([128, 128], f16, tag="pcs")
                                        nc.vector.tensor_copy(out=pcs[:], in_=pst[:])
                                        r0 = c0 + cc + c4
                                        for (sc, w_, roff) in pc["segs"]:
                                            nc.sync.dma_start(
                                                out=outs_t[blk][r0:r0 + 128,
                                                                roff:roff + w_],
                                                in_=pcs[:, sc:sc + w_])

            def pool_block(ph, s_dst, in_table, er_src):
                g = gs[s_dst]
                Qd = g.Q
                n_dst = NN[s_dst]
                with ExitStack() as ctx:
                    gp = ctx.enter_context(tc.tile_pool(name=f"plg{s_dst}", bufs=3))
                    pp = ctx.enter_context(
                        tc.tile_pool(name=f"plp{s_dst}", bufs=4, space="PSUM"))
                    stp = ctx.enter_context(tc.tile_pool(name=f"pls{s_dst}", bufs=1))
                    stg = stp.tile([128, Qd, er_src], f16)
                    grp = {32: 6, 16: 3}[s_dst]  # stripes per gather
                    for q0 in range(0, Qd, grp):
                        qn = min(grp, Qd - q0)
                        ns = qn * 4
                        gout = gp.tile([128, ns, er_src], f16, tag="plgout")
                        nc.gpsimd.dma_gather(
                            out_ap=gout[:], in_ap=in_table[:],
                            idxs_ap=idxpool_t[s_dst][:, q0 * 4 * 8:(q0 + qn) * 4 * 8],
                            num_idxs=ns * 128, num_idxs_reg=ns * 128,
                            elem_size=er_src, single_packet=False)
                        for qq in range(qn):
                            ps = pp.tile([128, er_src], f32, tag="plps")
                            for j in range(4):
                                nc.tensor.matmul(
                                    out=ps[:], lhsT=ident[:, 0:128],
                                    rhs=gout[:, qq * 4 + j, :],
                                    start=(j == 0), stop=(j == 3))
                            nc.vector.tensor_scalar_mul(stg[:, q0 + qq, :],
                                                        ps[:], 0.25)
                    nc.sync.dma_start(
                        out=pool_t[s_dst].rearrange("(p q) e -> p q e", p=128),
                        in_=stg[:])

            # ---------------- emit ----------------
            phases = {}
            for s in (64, 32, 16):
                g = gs[s]
                ph = phases[s] = {}
                ph["idxp_t"] = constp.tile([128, g.S * 8], i16, name=f"idxpt{s}")
                nc.sync.dma_start(out=ph["idxp_t"][:], in_=gh[s]["idxp"][:])
                ph["wl_t"] = constp.tile([128, g.S], f16, name=f"wlt{s}")
                nc.sync.dma_start(out=ph["wl_t"][:], in_=gh[s]["wl"][:])
                ph["wl2_t"] = constp.tile([128, g.S], f16, name=f"wl2t{s}")
                nc.sync.dma_start(out=ph["wl2_t"][:], in_=gh[s]["wl2"][:])
                ph["idxe_t"] = constp.tile([128, gs[s].n // 16], i16, name=f"idxet{s}")
                nc.sync.dma_start(out=ph["idxe_t"][:], in_=gh[s]["idxe"][:])
                ph["hubc_t"] = constp.tile([1, gs[s].n], f16, name=f"hubct{s}")
                nc.sync.dma_start(out=ph["hubc_t"][:], in_=gh[s]["hubc"][:])
                ph["hub_pos"] = gs[s].hub_pos
            idxpool_t = {}
            for s in (32, 16):
                idxpool_t[s] = constp.tile([128, gs[s].n // 4], i16,
                                           name=f"idxpoolt{s}")
                nc.sync.dma_start(out=idxpool_t[s][:], in_=idxpool[s][:])

            # x prep: build xorig then resort into block0 input staging
            with ExitStack() as ctx:
                xp = ctx.enter_context(tc.tile_pool(name="xp", bufs=1))
                xb = xp.tile([128, 96, 128], f16)
                nc.vector.memset(xb[:], 0.0)
                for b in range(4):
                    xf = xp.tile([128, 96], f32, tag=f"xf{b}")
                    nc.sync.dma_start(
                        out=xf[:],
                        in_=xin[b, :].rearrange("(p q) -> p q", p=128))
                    nc.vector.tensor_copy(out=xb[:, :, b], in_=xf[:])
                nc.sync.dma_start(
                    out=xorig.rearrange("(p q) e -> p q e", p=128), in_=xb[:])
                irt = xp.tile([128, 768], i16)
                nc.sync.dma_start(out=irt[:], in_=idxr[:])
                x0 = xp.tile([128, 96, 128], f16)
                nc.gpsimd.dma_gather(
                    out_ap=x0[:], in_ap=xorig[:], idxs_ap=irt[:],
                    num_idxs=12288, num_idxs_reg=12288, elem_size=128,
                    single_packet=False)
                # block0 input table
                b0in = nc.dram_tensor("b0in", [12288, 128], f16).ap()
                nc.sync.dma_start(
                    out=b0in.rearrange("(p q) e -> p q e", p=128), in_=x0[:])

            def _emit_head():
                with ExitStack() as ctx:
                    hp = ctx.enter_context(tc.tile_pool(name="head", bufs=1))
                    hpp = ctx.enter_context(
                        tc.tile_pool(name="headps", bufs=2, space="PSUM"))
                    w1 = hp.tile([128, 2, 2, 128], f32)
                    nc.sync.dma_start(out=w1[:], in_=wf1_h.rearrange("a b r m -> r a b m"))
                    b1 = hp.tile([128, 2], f32)
                    nc.sync.dma_start(out=b1[:], in_=bf1_h.rearrange("j r one -> r (j one)"))
                    w2 = hp.tile([128, 2, 128], f32)
                    nc.sync.dma_start(out=w2[:], in_=wf2_h.rearrange("a r m -> r a m"))
                    b2 = hp.tile([128, 1], f32)
                    nc.sync.dma_start(out=b2[:], in_=bf2_h[:])
                    wo = hp.tile([128, 1], f32)
                    nc.sync.dma_start(out=wo[:], in_=wo_h[:])
                    bo = hp.tile([1, 1], f32)
                    nc.sync.dma_start(out=bo[:], in_=bo_h[:])
                    accsum = [hp.tile([128, 4], f32, tag=f"as{a}",
                                      name=f"accsum{a}") for a in range(2)]
                    for a, acct in enumerate((acc0, acc1)):
                        nc.vector.tensor_add(out=accsum[a][:],
                                             in0=acct[:, :, 0],
                                             in1=acct[:, :, 1])
                    accs = accsum
                    z1 = [hp.tile([128, 4], f32, tag=f"z1_{b}", name=f"z1_{b}")
                          for b in range(2)]
                    for bb in range(2):
                        ps = hpp.tile([128, 4], f32, tag="hps")
                        for a in range(2):
                            nc.tensor.matmul(out=ps[:], lhsT=w1[:, a, bb, :],
                                             rhs=accs[a][:], start=(a == 0),
                                             stop=(a == 1))
                        nc.scalar.activation(out=z1[bb][:], in_=ps[:], func=AF.Relu,
                                             bias=b1[:, bb:bb + 1])
                    ps2 = hpp.tile([128, 4], f32, tag="hps2")
                    for a in range(2):
                        nc.tensor.matmul(out=ps2[:], lhsT=w2[:, a, :], rhs=z1[a][:],
                                         start=(a == 0), stop=(a == 1))
                    z2 = hp.tile([128, 4], f32)
                    nc.scalar.activation(out=z2[:], in_=ps2[:], func=AF.Relu,
                                         bias=b2[:, 0:1])
                    ps3 = hpp.tile([1, 4], f32, tag="hps3")
                    nc.tensor.matmul(out=ps3[:], lhsT=wo[:], rhs=z2[:],
                                     start=True, stop=True)
                    osb = hp.tile([1, 4], f32)
                    nc.scalar.activation(out=osb[:], in_=ps3[:], func=AF.Identity,
                                         bias=bo[:])
                    nc.sync.dma_start(out=out_d[:], in_=osb[:])

            stages = [
                lambda: conv_block(phases[64], 0, gs[64], 64, b0in, 1, 32),
                lambda: conv_block(phases[64], 1, gs[64], 64, outs_t[0], 32, 32),
                lambda: conv_block(phases[64], 2, gs[64], 64, outs_t[1], 32, 64),
                lambda: pool_block(phases[64], 32, outs_t[2], 256),
                lambda: conv_block(phases[32], 3, gs[32], 32, pool_t[32], 64, 64),
                lambda: conv_block(phases[32], 4, gs[32], 32, outs_t[3], 64, 128),
                lambda: pool_block(phases[32], 16, outs_t[4], 512),
                lambda: conv_block(phases[16], 5, gs[16], 16, pool_t[16], 128, 128),
                lambda: conv_block(phases[16], 6, gs[16], 16, outs_t[5], 128,
                                   256, last=True),
            ]
            for si, fn in enumerate(stages):
                if si < upto:
                    fn()
            if dump is not None:
                dt_map = {"b0in": b0in, "out0": outs_t[0], "out1": outs_t[1],
                          "out2": outs_t[2], "pool32": pool_t[32],
                          "out3": outs_t[3], "out4": outs_t[4],
                          "pool16": pool_t[16], "out5": outs_t[5],
                          "T0_1": tabs[0][0], "T1_1": tabs[1][0],
                          "T0_5": tabs[0][4]}
                src = dt_map[dump]
                rows, erow = src.shape
                dbg = nc.dram_tensor("dbg", [rows, erow], f16,
                                     kind="ExternalOutput").ap()
                with tc.tile_pool(name="dbgp", bufs=2) as dp:
                    for r0 in range(0, rows, 128):
                        t = dp.tile([128, erow], f16, tag="dbgt")
                        nc.sync.dma_start(out=t[:], in_=src[r0:r0 + 128, :])
                        nc.sync.dma_start(out=dbg[r0:r0 + 128, :], in_=t[:])

            if upto < 9:
                with tc.tile_pool(name="zo", bufs=1) as zp:
                    zt = zp.tile([1, 4], f32)
                    nc.vector.memset(zt[:], 0.0)
                    nc.sync.dma_start(out=out_d[:], in_=zt[:])

            # ---------------- head ----------------
            if upto >= 9:
                _emit_head()
    nc.compile()
    return nc, dev_in


_CACHE = {}


def kernel(**inputs):
    from concourse.bass_utils import run_bass_kernel_spmd

    inp = {k: np.asarray(v) for k, v in inputs.items()}
    st = _build_structs(inp)
    for k in ("Wf1", "bf1", "Wf2", "bf2", "Wo", "bo"):
        st[k] = inp[k]
    nc, dev_in = _build_bass(st)

    x = inp["x"].reshape(32, 12288).astype(np.float32)
    in_maps = []
    for c in range(8):
        m = dict(dev_in)
        m["xin"] = np.ascontiguousarray(x[4 * c:4 * (c + 1)])
        in_maps.append(m)
    res = run_bass_kernel_spmd(nc, in_maps, list(range(8)))
    out = np.zeros((32, 1), np.float32)
    for c in range(8):
        out[4 * c:4 * (c + 1), 0] = res.results[c]["out"][0]
    return out

